# revision 32
# baseline (speedup 1.0000x reference)
"""Trainium2 Bass kernel for multi-head causal self-attention.

Problem: B=4, T=1024, D=2048, H=16 heads, E=128 head_dim, fp32 I/O.
  q/k/v = per-head projections of x; scores = causal-masked softmax(q k^T / sqrt(E));
  y = probs @ v; out = concat-heads(y) @ Wo^T + bo.

Sharding: 8 cores = 4 batches x 2 head-halves. Core c handles batch c//2 and
heads (c%2)*8 .. (c%2)*8+7. Host sums the two half partials per batch and adds
the folded bias (bv folds through softmax into bo; bk is softmax-invariant).

Precision/throughput scheme:
  - Projections (A) and out-projection (C) run as fp8 DoubleRow matmuls at
    0.5 cycles/row with 2 contraction tiles packed per instruction. Full
    precision is recovered with a 3-term hi/lo decomposition:
        W@X ~= Whi@Xhi + Wlo@Xhi + Whi@Xlo
    where hi = e4m3(a), lo = a - hi (e5m2 for weights, whose lo falls below
    e4m3's subnormal range; e4m3 for activations). Cost is 0.75x of an
    fp32r/bf16 matmul at ~bf16-level accuracy.
  - q/k drop the x-lo correction term (softmax damps the ~1% score error;
    measured end-to-end rel_l2 is 1.43e-2 vs the 2e-2 budget). v keeps all
    three terms since its error passes straight to the output.
  - x and all weights are hi/lo-split on the host. y (out-proj input) is
    split on-device on DVE: yf = y*(1/r) fp32, yh = e4m3(yf), yl = yf - yh.
  - Attention (B) runs in float16 (exp outputs max ~e^2, far from overflow):
    scores k-stationary/q-moving pre-transposed per key tile so exp writes
    the P@V moving operand straight to SBUF; row sums ride as an all-ones
    matmul next to y; 1/r is applied at y eviction. v is produced already
    [t,e]-transposed by Phase A (x-tiles stationary), so Phase B needs no PE
    transposes. exp absorbs the 1/sqrt(E) scale. Heads are software-
    pipelined: the next head's first two score rows + exp run inside the
    current head's AV tail, and the chunk-1 AV stream lags chunk-0 by three
    k-tiles so the y-normalization DVE chain never gates the next head.
  - PSUM "zero region" rule: a matmul with start=True marks its whole 2KB
    region pending-zero, so sibling accumulation groups sharing a bank are
    opened by one full-bank zeros (x) zeros matmul instead (psv).
  - DMA: HWDGE costs ~625ns per dma_start (serialized), so transfers are
    batched coarsely in head-0 consumption order; out-proj weights preload
    during B; output partials are fp16 (summed on host in fp32).
"""

import numpy as np

B, T, D, H = 4, 1024, 2048, 16
E = D // H            # 128
P = 128
ND = D // P           # 16 d-tiles
NP = ND // 2          # 8 d-tile pairs (DoubleRow)
NT = T // P           # 8 t-blocks / q-blocks / k-tiles
HL = H // 2           # 8 heads per core
NPC = HL // 2         # 4 i-tile pairs in out-proj
SCALE = 1.0 / np.sqrt(E)

_cache = {}


def _build():
    import concourse.bass as bass
    import concourse.mybir as mybir
    import concourse.tile as tile
    from concourse import bacc
    from concourse.bass import ts

    F32 = mybir.dt.float32
    F16 = mybir.dt.float16
    BF16 = mybir.dt.bfloat16
    F8E4 = mybir.dt.float8e4
    F8E5 = mybir.dt.float8e5
    AF = mybir.ActivationFunctionType
    OP = mybir.AluOpType
    DR = mybir.MatmulPerfMode.DoubleRow

    nc = bacc.Bacc("TRN2", target_bir_lowering=False, debug=False)

    xh_d = nc.dram_tensor("xh", [P, ND, T], F8E4, kind="ExternalInput").ap()
    xl_d = nc.dram_tensor("xl", [P, ND, T], F8E4, kind="ExternalInput").ap()
    w3h_d = nc.dram_tensor("w3h", [HL, P, NP, 2, 3 * E], F8E4,
                           kind="ExternalInput").ap()
    w3l_d = nc.dram_tensor("w3l", [HL, P, NP, 2, 3 * E], F8E5,
                           kind="ExternalInput").ap()
    bqT_d = nc.dram_tensor("bqT", [P, HL], F32, kind="ExternalInput").ap()
    woth_d = nc.dram_tensor("woth", [P, ND, NPC, 2, P], F8E4,
                            kind="ExternalInput").ap()
    wotl_d = nc.dram_tensor("wotl", [P, ND, NPC, 2, P], F8E5,
                            kind="ExternalInput").ap()
    outT_d = nc.dram_tensor("outT", [ND, P, T], F16,
                            kind="ExternalOutput").ap()

    QS, KS, VS = slice(0, E), slice(E, 2 * E), slice(2 * E, 3 * E)

    with tile.TileContext(nc) as tc:
        with (
            tc.tile_pool(name="const", bufs=1) as const,
            tc.tile_pool(name="qkv", bufs=1) as qkv,
            tc.tile_pool(name="small", bufs=4) as small,
        ):
            ones_f = const.tile([P, P], F32)
            nc.vector.memset(ones_f[:], 1.0)
            ones_b = const.tile([P, P], F16)
            nc.vector.tensor_copy(ones_b[:], ones_f[:])
            bqT_t = const.tile([P, HL], F32)
            # all-zero fp8 operands: a full-bank "zero matmul" opens each psv
            # PSUM bank exactly once (one start per 2KB zero region; psv's
            # per-t-block groups then accumulate without further starts)
            zeros_t = const.tile([P, 2, 512], F8E4)
            nc.vector.memset(zeros_t[:], 0.0)

            qT = qkv.tile([P, HL, T], F16)           # [e, head, t]
            kT = qkv.tile([P, HL, T], F16)
            vA = qkv.tile([P, HL, NT, E], F16)       # [t-in-block, head, tb, e]
            yh = qkv.tile([P, HL, T], F8E4)          # y hi  [e, head, t]
            yl = qkv.tile([P, HL, T], F8E4)          # y lo

            # ---------------- Phase A: q/k/v projections (fp8 DR) ----------
            with (
                tc.tile_pool(name="xp", bufs=1) as xp,
                tc.tile_pool(name="w3p", bufs=2) as w3p,
                tc.tile_pool(name="ps_a", bufs=4, space="PSUM") as ps_a,
            ):
                xh_t = xp.tile([P, ND, T], F8E4)
                xl_t = xp.tile([P, ND, T], F8E4)

                def w3_dma(hl, split=False):
                    w3h_t = w3p.tile([P, NP, 2, 3 * E], F8E4, tag="w3h",
                                     name="w3h_t")
                    w3l_t = w3p.tile([P, NP, 2, 3 * E], F8E5, tag="w3l",
                                     name="w3l_t")
                    if not split:  # split: caller issues per-pair DMAs
                        nc.sync.dma_start(w3h_t[:], w3h_d[hl])
                        nc.sync.dma_start(w3l_t[:], w3l_d[hl])
                    return w3h_t, w3l_t

                # Head-0 DMA stream in consumption order (head 0 runs
                # pair-major main+wcorr, so stream (xh, w3h, w3l) per 2-pair
                # chunk). HWDGE charges ~625ns per dma_start, so chunks stay
                # coarse. h1's weights ride between the xl halves.
                w3_h0 = w3_dma(0, split=True)
                nc.sync.dma_start(xh_t[:, 0:2, 0:512], xh_d[:, 0:2, 0:512])
                nc.sync.dma_start(w3_h0[0][:, 0:2], w3h_d[0][:, 0:2])
                nc.sync.dma_start(xh_t[:, 0:2, 512:T], xh_d[:, 0:2, 512:T])
                nc.sync.dma_start(xh_t[:, 2:4, :], xh_d[:, 2:4, :])
                nc.sync.dma_start(w3_h0[1][:, 0:2], w3l_d[0][:, 0:2])
                for g in range(1, 4):
                    pl, ph = 2 * g, 2 * g + 2
                    nc.sync.dma_start(xh_t[:, 2 * pl:2 * ph, :],
                                      xh_d[:, 2 * pl:2 * ph, :])
                    nc.sync.dma_start(w3_h0[0][:, pl:ph], w3h_d[0][:, pl:ph])
                    nc.sync.dma_start(w3_h0[1][:, pl:ph], w3l_d[0][:, pl:ph])
                w3_h1 = (w3p.tile([P, NP, 2, 3 * E], F8E4, tag="w3h",
                                  name="w3h_t"),
                         w3p.tile([P, NP, 2, 3 * E], F8E5, tag="w3l",
                                  name="w3l_t"))
                nc.sync.dma_start(xl_t[:, 0:8, :], xl_d[:, 0:8, :])
                nc.sync.dma_start(w3_h1[0][:], w3h_d[1])
                nc.sync.dma_start(xl_t[:, 8:16, :], xl_d[:, 8:16, :])
                nc.sync.dma_start(w3_h1[1][:], w3l_d[1])
                nc.sync.dma_start(bqT_t[:], bqT_d)
                # preload the ACT Exp table off the critical path
                dummy = small.tile([P, 1], F32, tag="racc", name="dummy")
                nc.scalar.activation(dummy[:], bqT_t[:, 0:1], AF.Exp)

                defer_vx = [None]
                for hl in range(HL):
                    if hl == 0:
                        w3h_t, w3l_t = w3_h0
                    elif hl == 1:
                        w3h_t, w3l_t = w3_h1
                    else:
                        w3h_t, w3l_t = w3_nx[0]
                    psq = ps_a.tile([P, T], F32, tag="a")
                    psk = ps_a.tile([P, T], F32, tag="a")
                    psv = ps_a.tile([P, NT, E], F32, tag="a")
                    for bank in range(2):
                        nc.tensor.matmul(
                            psv[:, 4 * bank:4 * bank + 4, :],
                            zeros_t[:, :, 0:P], zeros_t[:],
                            start=True, stop=False, perf_mode=DR,
                            skip_group_check=True)

                    # terms: (stationary-w, moving-x) for q/k;
                    #        (stationary-x, moving-w) for v
                    terms = [(w3h_t, xh_t), (w3l_t, xh_t), (w3h_t, xl_t)]

                    # q/k skip the x-lo correction term (softmax damps the
                    # resulting ~1% score error; v keeps all 3 terms since v
                    # error passes straight through to the output)
                    def emit_qk(wt, xt, pr, s0, sqk):
                        xpair = xt[:, 2 * pr:2 * pr + 2, :]
                        for c in range(2):
                            nc.tensor.matmul(
                                psq[:, ts(c, 512)], wt[:, pr, :, QS],
                                xpair[:, :, ts(c, 512)], start=s0, stop=sqk,
                                perf_mode=DR, skip_group_check=True)
                            nc.tensor.matmul(
                                psk[:, ts(c, 512)], wt[:, pr, :, KS],
                                xpair[:, :, ts(c, 512)], start=s0, stop=sqk,
                                perf_mode=DR, skip_group_check=True)

                    def emit_vpr(wt, xt, pr, sv, psv=psv):
                        xpair = xt[:, 2 * pr:2 * pr + 2, :]
                        for tb in range(NT):
                            nc.tensor.matmul(
                                psv[:, tb, :], xpair[:, :, ts(tb, P)],
                                wt[:, pr, :, VS], start=False,
                                stop=sv and tb % 4 == 3,
                                perf_mode=DR, skip_group_check=True)

                    if hl == 0:
                        # pair-major main+wcorr so the work per pair matches
                        # the (xh, w3h, w3l) DMA supply per pair
                        for pr in range(NP):
                            emit_qk(w3h_t, xh_t, pr, pr == 0, False)
                            emit_vpr(w3h_t, xh_t, pr, False)
                            emit_qk(w3l_t, xh_t, pr, False, pr == NP - 1)
                            emit_vpr(w3l_t, xh_t, pr, False)
                        # v-xcorr deferred until xl has landed (into head 1)
                    else:
                        for ti, (wt, xt) in enumerate(terms):
                            if ti < 2:
                                for pr in range(NP):
                                    emit_qk(wt, xt, pr,
                                            ti == 0 and pr == 0,
                                            ti == 1 and pr == NP - 1)
                            for pr in range(NP):
                                emit_vpr(wt, xt, pr, ti == 2 and pr == NP - 1)
                            if hl == 1 and ti == 0 and defer_vx[0] is not None:
                                defer_vx[0]()
                                defer_vx[0] = None

                    # prefetch next head's weights while this head computes
                    if 1 <= hl < HL - 1:
                        w3_nx = [w3_dma(hl + 1)]

                    # evict: q gets +bq on DVE; k on ACT; v on DVE (bf16 out)
                    if hl == 0:
                        nc.vector.tensor_scalar(
                            qT[:, hl, :], psq[:], bqT_t[:, hl:hl + 1], None,
                            op0=OP.add)
                        nc.scalar.activation(kT[:, hl, :], psk[:], AF.Copy)

                        def _vx(w3h_t=w3h_t, psv=psv, emit_vpr=emit_vpr):
                            for pr in range(NP):
                                emit_vpr(w3h_t, xl_t, pr, pr == NP - 1,
                                         psv=psv)
                            nc.vector.tensor_copy(vA[:, 0, 0:NT // 2],
                                                  psv[:, 0:NT // 2])
                            nc.scalar.activation(vA[:, 0, NT // 2:NT],
                                                 psv[:, NT // 2:NT], AF.Copy)
                        defer_vx[0] = _vx
                    elif hl < HL - 1:
                        nc.vector.tensor_scalar(
                            qT[:, hl, :], psq[:], bqT_t[:, hl:hl + 1], None,
                            op0=OP.add)
                        nc.scalar.activation(kT[:, hl, :], psk[:], AF.Copy)
                        nc.vector.tensor_copy(vA[:, hl, 0:NT // 2],
                                              psv[:, 0:NT // 2])
                        nc.scalar.activation(vA[:, hl, NT // 2:NT],
                                             psv[:, NT // 2:NT], AF.Copy)
                    else:
                        # last head: fine-grained two-engine evictions so the
                        # A->B pool-transition drain has a short tail
                        nc.vector.tensor_scalar(
                            qT[:, hl, 0:512], psq[:, 0:512],
                            bqT_t[:, hl:hl + 1], None, op0=OP.add)
                        nc.scalar.activation(kT[:, hl, 0:512], psk[:, 0:512],
                                             AF.Copy)
                        nc.vector.tensor_scalar(
                            qT[:, hl, 512:T], psq[:, 512:T],
                            bqT_t[:, hl:hl + 1], None, op0=OP.add)
                        nc.scalar.activation(kT[:, hl, 512:T], psk[:, 512:T],
                                             AF.Copy)
                        nc.vector.tensor_copy(vA[:, hl, 0:NT // 2],
                                              psv[:, 0:NT // 2])
                        nc.scalar.activation(vA[:, hl, NT // 2:NT],
                                             psv[:, NT // 2:NT], AF.Copy)

            # ---------------- Phases B+C scope ----------------
            with (
                tc.tile_pool(name="wop", bufs=1) as wop,
                tc.tile_pool(name="rbp", bufs=2) as rbp,
                tc.tile_pool(name="yfp", bufs=2) as yfp,
                tc.tile_pool(name="ps_ar", bufs=2, space="PSUM") as ps_ar,
            ):
                # preload all out-proj weights in two DMAs during B (the
                # DMA + HWDGE engines are otherwise idle there)
                woth_t = wop.tile([P, ND, NPC, 2, P], F8E4, name="woth_t")
                wotl_t = wop.tile([P, ND, NPC, 2, P], F8E5, name="wotl_t")
                nc.sync.dma_start(woth_t[:], woth_d)
                nc.sync.dma_start(wotl_t[:], wotl_d)

                # -------- Phase B: attention per head (bf16), software-
                # pipelined across heads: the next head's first two score
                # rows (and their exp) are emitted inside the current head's
                # AV tail, and the chunk-1 AV stream lags chunk-0 by one
                # k-tile so the y-normalization DVE chain never gates the
                # next head's first AV matmuls. --------
                with (
                    tc.tile_pool(name="etp", bufs=2) as etp,
                    tc.tile_pool(name="ps_s", bufs=2, space="PSUM") as ps_s,
                ):
                    def make_head(hl):
                        return {
                            "hl": hl,
                            "ET": etp.tile([P, NT, T], F16, tag="ET", name="ET"),
                            "rb": rbp.tile([P, T], F32, tag="rb", name="rb"),
                            "y0": ps_ar.tile([P, 512], F32, tag="y", name="y0"),
                            "r0": ps_ar.tile([P, 512], F32, tag="r", name="r0"),
                            "y1": ps_ar.tile([P, 512], F32, tag="y", name="y1"),
                            "r1": ps_ar.tile([P, 512], F32, tag="r", name="r1"),
                        }

                    def emit_ST(h, j):
                        # scores for k-tile j: per-q-chunk 1-bank psums (s0:
                        # q<512, s1: q>=512) keep the ST pipeline deep and the
                        # AV0 exp dependency short
                        hl, ET = h["hl"], h["ET"]
                        kblk = kT[:, hl, ts(j, P)]
                        if j < 4:
                            s0 = ps_s.tile([P, 512], F32, tag="s0", name="s0")
                            nc.tensor.matmul(s0[:, j * P:512], kblk,
                                             qT[:, hl, j * P:512],
                                             start=True, stop=True)
                            s1 = ps_s.tile([P, 512], F32, tag="s1", name="s1")
                            nc.tensor.matmul(s1[:], kblk, qT[:, hl, 512:T],
                                             start=True, stop=True)
                            nc.scalar.activation(ET[:, j, j * P:512],
                                                 s0[:, j * P:512], AF.Exp,
                                                 scale=float(SCALE))
                            nc.scalar.activation(ET[:, j, 512:T], s1[:],
                                                 AF.Exp, scale=float(SCALE))
                        else:
                            lo = j * P - 512
                            s1 = ps_s.tile([P, 512], F32, tag="s1", name="s1")
                            nc.tensor.matmul(s1[:, lo:512], kblk,
                                             qT[:, hl, j * P:T],
                                             start=True, stop=True)
                            nc.scalar.activation(ET[:, j, j * P:T],
                                                 s1[:, lo:512], AF.Exp,
                                                 scale=float(SCALE))
                        nc.gpsimd.affine_select(
                            out=ET[:, j, j * P:(j + 1) * P],
                            in_=ET[:, j, j * P:(j + 1) * P],
                            compare_op=OP.is_ge, fill=0.0,
                            base=0, pattern=[[1, P]], channel_multiplier=-1,
                        )

                    def emit_AV0(h, jq):  # q-chunk 0: cols jq*P..512, jq<=3
                        lo = jq * P
                        st, sp = jq == 0, jq == 3
                        ET = h["ET"]
                        nc.tensor.matmul(h["y0"][:, lo:512],
                                         vA[:, h["hl"], jq, :],
                                         ET[:, jq, lo:512], start=st, stop=sp,
                                         skip_group_check=True)
                        nc.tensor.matmul(h["r0"][:, lo:512], ones_b[:],
                                         ET[:, jq, lo:512], start=st, stop=sp,
                                         skip_group_check=True)

                    def emit_AV1(h, jq):  # q-chunk 1: cols 512..T, all jq
                        lo = max(jq * P, 512)
                        st, sp = jq == 0, jq == NT - 1
                        ET = h["ET"]
                        nc.tensor.matmul(h["y1"][:, lo - 512:512],
                                         vA[:, h["hl"], jq, :],
                                         ET[:, jq, lo:T], start=st, stop=sp,
                                         skip_group_check=True)
                        nc.tensor.matmul(h["r1"][:, lo - 512:512], ones_b[:],
                                         ET[:, jq, lo:T], start=st, stop=sp,
                                         skip_group_check=True)

                    def emit_ynorm(h, c, y, r, halves=False):
                        # yf = y * (1/r) f32; hi = e4m3(yf); lo = yf - hi
                        # (all on DVE: ACT is the busier engine in B)
                        hl, rb = h["hl"], h["rb"]
                        yf = yfp.tile([P, 512], F32, tag="yf", name="yf")
                        nsp = 1
                        w = 512 // nsp
                        for u in range(nsp):
                            cl = slice(c * 512 + u * w, c * 512 + (u + 1) * w)
                            ul = slice(u * w, (u + 1) * w)
                            nc.vector.reciprocal(rb[:, cl], r[:, ul])
                            nc.vector.tensor_mul(yf[:, ul], y[:, ul], rb[:, cl])
                            nc.vector.tensor_copy(yh[:, hl, cl], yf[:, ul])
                            nc.vector.tensor_tensor(
                                yl[:, hl, cl], yf[:, ul],
                                yh[:, hl, cl], op=OP.subtract)

                    cur = make_head(0)
                    emit_ST(cur, 0)
                    emit_ST(cur, 1)
                    emit_ST(cur, 2)
                    for hl in range(HL):
                        for j in range(2, NT):
                            if not (hl == 0 and j == 2):
                                emit_ST(cur, j)
                            if j - 2 <= 3:
                                emit_AV0(cur, j - 2)
                            if j >= 5:
                                emit_AV1(cur, j - 5)
                            if j == 5:
                                emit_ynorm(cur, 0, cur["y0"], cur["r0"])
                        emit_AV1(cur, 3)
                        emit_AV1(cur, 4)
                        if hl + 1 < HL:
                            nxt = make_head(hl + 1)
                            emit_ST(nxt, 0)
                            emit_ST(nxt, 1)
                        emit_AV1(cur, 5)
                        emit_AV1(cur, 6)
                        emit_AV1(cur, 7)
                        if hl + 1 < HL:
                            emit_ynorm(cur, 1, cur["y1"], cur["r1"])
                            cur = nxt
                        else:
                            emit_ynorm(cur, 1, cur["y1"], cur["r1"],
                                       halves=True)

                # -------- Phase C: partial out-projection (fp8 DR) --------
                with (
                    tc.tile_pool(name="osb", bufs=3) as osb,
                    tc.tile_pool(name="ps_o", bufs=2, space="PSUM") as ps_o,
                ):
                    for ob in range(ND):
                        o_ps = ps_o.tile([P, T], F32, tag="o")
                        out_sb = osb.tile([P, T], F16, tag="osb")
                        # last block's second chunk runs as two half-width
                        # psum groups so its first eviction+DMA overlaps the
                        # final matmuls (shortens the end drain)
                        subs = ([(0, 512), (512, 768), (768, 1024)]
                                if ob == ND - 1 else [(0, 512), (512, 1024)])
                        for si, (lo, hi) in enumerate(subs):
                            n = 0
                            nsub = 3 * NPC
                            for wt, yt in ((woth_t, yh), (wotl_t, yh),
                                           (woth_t, yl)):
                                for pr in range(NPC):
                                    nc.tensor.matmul(
                                        o_ps[:, lo:hi], wt[:, ob, pr],
                                        yt[:, 2 * pr:2 * pr + 2, lo:hi],
                                        start=n == 0, stop=n == nsub - 1,
                                        perf_mode=DR, skip_group_check=True)
                                    n += 1
                            # evict + DMA per sub-chunk; alternate engines
                            if si == 0:
                                nc.scalar.activation(out_sb[:, lo:hi],
                                                     o_ps[:, lo:hi], AF.Copy)
                            else:
                                nc.vector.tensor_copy(out_sb[:, lo:hi],
                                                      o_ps[:, lo:hi])
                            nc.sync.dma_start(outT_d[ob][:, lo:hi],
                                              out_sb[:, lo:hi])

    nc.compile()
    return nc


def _get_compiled():
    if "nc" not in _cache:
        _cache["nc"] = _build()
    return _cache["nc"]


def _hilo(a, lo_dt):
    import ml_dtypes
    hi = np.ascontiguousarray(a).astype(ml_dtypes.float8_e4m3)
    lo = (a - hi.astype(np.float32)).astype(lo_dt)
    return hi, lo


def _host_prep(x, Wq, bq, Wk, Wv, Wo):
    """Build per-core input maps (hi/lo fp8 splits + DR pair packing)."""
    import ml_dtypes
    E4, E5 = ml_dtypes.float8_e4m3, ml_dtypes.float8_e5m2
    in_maps = []
    xs = []
    for b in range(B):
        # [P, ND, T]: row p holds d-tile-major slices, matching the SBUF tile
        xT = np.ascontiguousarray(
            x[b].T.reshape(ND, P, T).transpose(1, 0, 2))
        xs.append(_hilo(xT, E4))
    halves = []
    for half in range(2):
        hs = slice(half * HL, (half + 1) * HL)
        # w3 packed [HL, P, NP, 2, 3E]: slot s of pair pr = d-tile 2pr+s
        w3 = np.empty((HL, P, NP, 2, 3 * E), dtype=np.float32)
        for hl, h in enumerate(range(half * HL, (half + 1) * HL)):
            for j, W in enumerate((Wq[h], Wk[h], Wv[h])):
                wt = W.T.reshape(NP, 2, P, E)          # [pr, s, p(d), e]
                w3[hl, :, :, :, j * E:(j + 1) * E] = wt.transpose(2, 0, 1, 3)
        w3h, w3l = _hilo(w3, E5)
        bqT = np.ascontiguousarray(bq[hs].T)           # [E, HL]
        # wot [ND, P, NPC, 2, P]: [ob, i-in-tile, pr, s, o], i-tile = 2pr+s
        WoT_span = Wo.T[half * 1024:(half + 1) * 1024]  # [1024, D]
        wot = np.ascontiguousarray(
            WoT_span.reshape(NPC, 2, P, ND, P).transpose(2, 3, 0, 1, 4))
        woth, wotl = _hilo(wot, E5)
        halves.append({"w3h": w3h, "w3l": w3l, "bqT": bqT,
                       "woth": woth, "wotl": wotl})
    for c in range(8):
        b, half = c // 2, c % 2
        hv = halves[half]
        in_maps.append({"xh": xs[b][0], "xl": xs[b][1], **hv})
    return in_maps


def _numpy_fallback(x, attention_mask, Wq, bq, Wk, bk, Wv, bv, Wo, bo):
    out = np.empty((B, T, D), dtype=np.float32)
    neg = np.float32(np.finfo(np.float32).min)
    for b in range(B):
        xb = x[b]
        q = np.einsum("td,hed->hte", xb, Wq) + bq[:, None, :]
        k = np.einsum("td,hed->hte", xb, Wk) + bk[:, None, :]
        v = np.einsum("td,hed->hte", xb, Wv) + bv[:, None, :]
        s = np.einsum("hqe,hke->hqk", q, k).astype(np.float32) * np.float32(SCALE)
        causal = np.arange(T)[None, :] > np.arange(T)[:, None]
        s = np.where(causal[None], neg, s)
        keep = attention_mask[b].astype(bool)
        s = np.where(keep[None, None, :], s, neg)
        s = s - s.max(-1, keepdims=True)
        p = np.exp(s)
        p = p / p.sum(-1, keepdims=True)
        y = np.einsum("hqk,hke->hqe", p, v)
        y = np.transpose(y, (1, 0, 2)).reshape(T, D)
        out[b] = y @ Wo.T + bo
    return out


def kernel(x, attention_mask, Wq, bq, Wk, bk, Wv, bv, Wo, bo):
    x = np.asarray(x, dtype=np.float32)
    attention_mask = np.asarray(attention_mask)
    Wq, bq = np.asarray(Wq, np.float32), np.asarray(bq, np.float32)
    Wk, bk = np.asarray(Wk, np.float32), np.asarray(bk, np.float32)
    Wv, bv = np.asarray(Wv, np.float32), np.asarray(bv, np.float32)
    Wo, bo = np.asarray(Wo, np.float32), np.asarray(bo, np.float32)

    if not np.all(attention_mask == 1):
        return _numpy_fallback(x, attention_mask, Wq, bq, Wk, bk, Wv, bv, Wo, bo)

    from concourse.bass_utils import run_bass_kernel_spmd

    nc = _get_compiled()
    in_maps = _host_prep(x, Wq, bq, Wk, Wv, Wo)
    res = run_bass_kernel_spmd(nc, in_maps, core_ids=list(range(8)))

    # bv folds through softmax (rows sum to 1); bk is softmax-invariant
    bo_total = (bo + Wo @ bv.reshape(D)).astype(np.float32)

    out = np.zeros((B, T, D), dtype=np.float32)
    for c in range(8):
        # fp16 partials off-device; summed here in fp32
        partial = res.results[c]["outT"].astype(np.float32).reshape(D, T)
        out[c // 2] += partial.T
    out += bo_total
    return out


# revision 33
# speedup vs baseline: 1.0054x; 1.0054x over previous
"""Trainium2 Bass kernel for multi-head causal self-attention.

Problem: B=4, T=1024, D=2048, H=16 heads, E=128 head_dim, fp32 I/O.
  q/k/v = per-head projections of x; scores = causal-masked softmax(q k^T / sqrt(E));
  y = probs @ v; out = concat-heads(y) @ Wo^T + bo.

Sharding: 8 cores = 4 batches x 2 head-halves. Core c handles batch c//2 and
heads (c%2)*8 .. (c%2)*8+7. Host sums the two half partials per batch and adds
the folded bias (bv folds through softmax into bo; bk is softmax-invariant).

Precision/throughput scheme:
  - Projections (A) and out-projection (C) run as fp8 DoubleRow matmuls at
    0.5 cycles/row with 2 contraction tiles packed per instruction. Full
    precision is recovered with a 3-term hi/lo decomposition:
        W@X ~= Whi@Xhi + Wlo@Xhi + Whi@Xlo
    where hi = e4m3(a), lo = a - hi (e5m2 for weights, whose lo falls below
    e4m3's subnormal range; e4m3 for activations). Cost is 0.75x of an
    fp32r/bf16 matmul at ~bf16-level accuracy.
  - q/k drop the x-lo correction term (softmax damps the ~1% score error;
    measured end-to-end rel_l2 is 1.43e-2 vs the 2e-2 budget). v keeps all
    three terms since its error passes straight to the output.
  - x and all weights are hi/lo-split on the host. y (out-proj input) is
    split on-device on DVE: yf = y*(1/r) fp32, yh = e4m3(yf), yl = yf - yh.
  - Attention (B) runs in float16 (exp outputs max ~e^2, far from overflow):
    scores k-stationary/q-moving pre-transposed per key tile so exp writes
    the P@V moving operand straight to SBUF; row sums ride as an all-ones
    matmul next to y; 1/r is applied at y eviction. v is produced already
    [t,e]-transposed by Phase A (x-tiles stationary), so Phase B needs no PE
    transposes. exp absorbs the 1/sqrt(E) scale. Heads are software-
    pipelined: the next head's first two score rows + exp run inside the
    current head's AV tail, and the chunk-1 AV stream lags chunk-0 by three
    k-tiles so the y-normalization DVE chain never gates the next head.
  - PSUM "zero region" rule: a matmul with start=True marks its whole 2KB
    region pending-zero, so sibling accumulation groups sharing a bank are
    opened by one full-bank zeros (x) zeros matmul instead (psv).
  - DMA: HWDGE costs ~625ns per dma_start (serialized), so transfers are
    batched coarsely in head-0 consumption order; out-proj weights preload
    during B; output partials are fp16 (summed on host in fp32).
"""

import numpy as np

B, T, D, H = 4, 1024, 2048, 16
E = D // H            # 128
P = 128
ND = D // P           # 16 d-tiles
NP = ND // 2          # 8 d-tile pairs (DoubleRow)
NT = T // P           # 8 t-blocks / q-blocks / k-tiles
HL = H // 2           # 8 heads per core
NPC = HL // 2         # 4 i-tile pairs in out-proj
SCALE = 1.0 / np.sqrt(E)

_cache = {}


def _build():
    import concourse.bass as bass
    import concourse.mybir as mybir
    import concourse.tile as tile
    from concourse import bacc
    from concourse.bass import ts

    F32 = mybir.dt.float32
    F16 = mybir.dt.float16
    BF16 = mybir.dt.bfloat16
    F8E4 = mybir.dt.float8e4
    F8E5 = mybir.dt.float8e5
    AF = mybir.ActivationFunctionType
    OP = mybir.AluOpType
    DR = mybir.MatmulPerfMode.DoubleRow

    nc = bacc.Bacc("TRN2", target_bir_lowering=False, debug=False)

    xh_d = nc.dram_tensor("xh", [P, ND, T], F8E4, kind="ExternalInput").ap()
    xl_d = nc.dram_tensor("xl", [P, ND, T], F8E4, kind="ExternalInput").ap()
    w3h_d = nc.dram_tensor("w3h", [HL, P, NP, 2, 3 * E], F8E4,
                           kind="ExternalInput").ap()
    w3l_d = nc.dram_tensor("w3l", [HL, P, NP, 2, 3 * E], F8E5,
                           kind="ExternalInput").ap()
    bqT_d = nc.dram_tensor("bqT", [P, HL], F32, kind="ExternalInput").ap()
    woth_d = nc.dram_tensor("woth", [P, ND, NPC, 2, P], F8E4,
                            kind="ExternalInput").ap()
    wotl_d = nc.dram_tensor("wotl", [P, ND, NPC, 2, P], F8E5,
                            kind="ExternalInput").ap()
    outT_d = nc.dram_tensor("outT", [ND, P, T], F16,
                            kind="ExternalOutput").ap()

    QS, KS, VS = slice(0, E), slice(E, 2 * E), slice(2 * E, 3 * E)

    with tile.TileContext(nc) as tc:
        with (
            tc.tile_pool(name="const", bufs=1) as const,
            tc.tile_pool(name="qkv", bufs=1) as qkv,
            tc.tile_pool(name="small", bufs=4) as small,
        ):
            ones_f = const.tile([P, P], F32)
            nc.vector.memset(ones_f[:], 1.0)
            ones_b = const.tile([P, P], F16)
            nc.vector.tensor_copy(ones_b[:], ones_f[:])
            bqT_t = const.tile([P, HL], F32)
            # all-zero fp8 operands: a full-bank "zero matmul" opens each psv
            # PSUM bank exactly once (one start per 2KB zero region; psv's
            # per-t-block groups then accumulate without further starts)
            zeros_t = const.tile([P, 2, 512], F8E4)
            nc.vector.memset(zeros_t[:], 0.0)

            qT = qkv.tile([P, HL, T], F16)           # [e, head, t]
            kT = qkv.tile([P, HL, T], F16)
            vA = qkv.tile([P, HL, NT, E], F16)       # [t-in-block, head, tb, e]
            yh = qkv.tile([P, HL, T], F8E4)          # y hi  [e, head, t]
            yl = qkv.tile([P, HL, T], F8E4)          # y lo

            # ---------------- Phase A: q/k/v projections (fp8 DR) ----------
            with (
                tc.tile_pool(name="xp", bufs=1) as xp,
                tc.tile_pool(name="w3p", bufs=2) as w3p,
                tc.tile_pool(name="ps_a", bufs=4, space="PSUM") as ps_a,
            ):
                xh_t = xp.tile([P, ND, T], F8E4)
                xl_t = xp.tile([P, ND, T], F8E4)

                def w3_dma(hl, split=False):
                    w3h_t = w3p.tile([P, NP, 2, 3 * E], F8E4, tag="w3h",
                                     name="w3h_t")
                    w3l_t = w3p.tile([P, NP, 2, 3 * E], F8E5, tag="w3l",
                                     name="w3l_t")
                    if not split:  # split: caller issues per-pair DMAs
                        nc.sync.dma_start(w3h_t[:], w3h_d[hl])
                        nc.sync.dma_start(w3l_t[:], w3l_d[hl])
                    return w3h_t, w3l_t

                # Head-0 DMA stream in consumption order (head 0 runs
                # pair-major main+wcorr, so stream (xh, w3h, w3l) per 2-pair
                # chunk). HWDGE charges ~625ns per dma_start, so chunks stay
                # coarse. h1's weights ride between the xl halves.
                w3_h0 = w3_dma(0, split=True)
                nc.sync.dma_start(xh_t[:, 0:2, 0:512], xh_d[:, 0:2, 0:512])
                nc.sync.dma_start(w3_h0[0][:, 0:2], w3h_d[0][:, 0:2])
                nc.sync.dma_start(xh_t[:, 0:2, 512:T], xh_d[:, 0:2, 512:T])
                nc.sync.dma_start(xh_t[:, 2:4, :], xh_d[:, 2:4, :])
                nc.sync.dma_start(w3_h0[1][:, 0:2], w3l_d[0][:, 0:2])
                for g in range(1, 4):
                    pl, ph = 2 * g, 2 * g + 2
                    nc.sync.dma_start(xh_t[:, 2 * pl:2 * ph, :],
                                      xh_d[:, 2 * pl:2 * ph, :])
                    nc.sync.dma_start(w3_h0[0][:, pl:ph], w3h_d[0][:, pl:ph])
                    nc.sync.dma_start(w3_h0[1][:, pl:ph], w3l_d[0][:, pl:ph])
                w3_h1 = (w3p.tile([P, NP, 2, 3 * E], F8E4, tag="w3h",
                                  name="w3h_t"),
                         w3p.tile([P, NP, 2, 3 * E], F8E5, tag="w3l",
                                  name="w3l_t"))
                nc.sync.dma_start(xl_t[:, 0:8, :], xl_d[:, 0:8, :])
                nc.sync.dma_start(w3_h1[0][:], w3h_d[1])
                nc.sync.dma_start(xl_t[:, 8:16, :], xl_d[:, 8:16, :])
                nc.sync.dma_start(w3_h1[1][:], w3l_d[1])
                nc.sync.dma_start(bqT_t[:], bqT_d)
                # preload the ACT Exp table off the critical path
                dummy = small.tile([P, 1], F32, tag="racc", name="dummy")
                nc.scalar.activation(dummy[:], bqT_t[:, 0:1], AF.Exp)

                defer_vx = [None]
                for hl in range(HL):
                    if hl == 0:
                        w3h_t, w3l_t = w3_h0
                    elif hl == 1:
                        w3h_t, w3l_t = w3_h1
                    else:
                        w3h_t, w3l_t = w3_nx[0]
                    psq = ps_a.tile([P, T], F32, tag="a")
                    psk = ps_a.tile([P, T], F32, tag="a")
                    psv = ps_a.tile([P, NT, E], F32, tag="a")
                    for bank in range(2):
                        nc.tensor.matmul(
                            psv[:, 4 * bank:4 * bank + 4, :],
                            zeros_t[:, :, 0:P], zeros_t[:],
                            start=True, stop=False, perf_mode=DR,
                            skip_group_check=True)

                    # terms: (stationary-w, moving-x) for q/k;
                    #        (stationary-x, moving-w) for v
                    terms = [(w3h_t, xh_t), (w3l_t, xh_t), (w3h_t, xl_t)]

                    # q/k skip the x-lo correction term (softmax damps the
                    # resulting ~1% score error; v keeps all 3 terms since v
                    # error passes straight through to the output)
                    def emit_qk(wt, xt, pr, s0, sqk):
                        xpair = xt[:, 2 * pr:2 * pr + 2, :]
                        for c in range(2):
                            nc.tensor.matmul(
                                psq[:, ts(c, 512)], wt[:, pr, :, QS],
                                xpair[:, :, ts(c, 512)], start=s0, stop=sqk,
                                perf_mode=DR, skip_group_check=True)
                            nc.tensor.matmul(
                                psk[:, ts(c, 512)], wt[:, pr, :, KS],
                                xpair[:, :, ts(c, 512)], start=s0, stop=sqk,
                                perf_mode=DR, skip_group_check=True)

                    def emit_vpr(wt, xt, pr, sv, psv=psv):
                        xpair = xt[:, 2 * pr:2 * pr + 2, :]
                        for tb in range(NT):
                            nc.tensor.matmul(
                                psv[:, tb, :], xpair[:, :, ts(tb, P)],
                                wt[:, pr, :, VS], start=False,
                                stop=sv and tb % 4 == 3,
                                perf_mode=DR, skip_group_check=True)

                    if hl == 0:
                        # pair-major main+wcorr so the work per pair matches
                        # the (xh, w3h, w3l) DMA supply per pair
                        for pr in range(NP):
                            emit_qk(w3h_t, xh_t, pr, pr == 0, False)
                            emit_vpr(w3h_t, xh_t, pr, False)
                            emit_qk(w3l_t, xh_t, pr, False, pr == NP - 1)
                            emit_vpr(w3l_t, xh_t, pr, False)
                        # v-xcorr deferred until xl has landed (into head 1)
                    else:
                        for ti, (wt, xt) in enumerate(terms):
                            if ti < 2:
                                for pr in range(NP):
                                    emit_qk(wt, xt, pr,
                                            ti == 0 and pr == 0,
                                            ti == 1 and pr == NP - 1)
                            for pr in range(NP):
                                emit_vpr(wt, xt, pr, ti == 2 and pr == NP - 1)
                            if hl == 1 and ti == 0 and defer_vx[0] is not None:
                                defer_vx[0]()
                                defer_vx[0] = None

                    # prefetch next head's weights while this head computes
                    if 1 <= hl < HL - 1:
                        w3_nx = [w3_dma(hl + 1)]

                    # evict: q gets +bq on DVE; k on ACT; v on DVE (bf16 out)
                    if hl == 0:
                        nc.vector.tensor_scalar(
                            qT[:, hl, :], psq[:], bqT_t[:, hl:hl + 1], None,
                            op0=OP.add)
                        nc.scalar.activation(kT[:, hl, :], psk[:], AF.Copy)

                        def _vx(w3h_t=w3h_t, psv=psv, emit_vpr=emit_vpr):
                            for pr in range(NP):
                                emit_vpr(w3h_t, xl_t, pr, pr == NP - 1,
                                         psv=psv)
                            nc.vector.tensor_copy(vA[:, 0, 0:NT // 2],
                                                  psv[:, 0:NT // 2])
                            nc.scalar.activation(vA[:, 0, NT // 2:NT],
                                                 psv[:, NT // 2:NT], AF.Copy)
                        defer_vx[0] = _vx
                    elif hl < HL - 1:
                        nc.vector.tensor_scalar(
                            qT[:, hl, :], psq[:], bqT_t[:, hl:hl + 1], None,
                            op0=OP.add)
                        nc.scalar.activation(kT[:, hl, :], psk[:], AF.Copy)
                        nc.vector.tensor_copy(vA[:, hl, 0:NT // 2],
                                              psv[:, 0:NT // 2])
                        nc.scalar.activation(vA[:, hl, NT // 2:NT],
                                             psv[:, NT // 2:NT], AF.Copy)
                    else:
                        # last head: fine-grained two-engine evictions so the
                        # A->B pool-transition drain has a short tail
                        nc.vector.tensor_scalar(
                            qT[:, hl, 0:512], psq[:, 0:512],
                            bqT_t[:, hl:hl + 1], None, op0=OP.add)
                        nc.scalar.activation(kT[:, hl, 0:512], psk[:, 0:512],
                                             AF.Copy)
                        nc.vector.tensor_scalar(
                            qT[:, hl, 512:T], psq[:, 512:T],
                            bqT_t[:, hl:hl + 1], None, op0=OP.add)
                        nc.scalar.activation(kT[:, hl, 512:T], psk[:, 512:T],
                                             AF.Copy)
                        nc.vector.tensor_copy(vA[:, hl, 0:NT // 2],
                                              psv[:, 0:NT // 2])
                        nc.scalar.activation(vA[:, hl, NT // 2:NT],
                                             psv[:, NT // 2:NT], AF.Copy)

            # ---------------- Phases B+C scope ----------------
            with (
                tc.tile_pool(name="wop", bufs=1) as wop,
            ):
                # preload all out-proj weights in two DMAs during B (the
                # DMA + HWDGE engines are otherwise idle there)
                woth_t = wop.tile([P, ND, NPC, 2, P], F8E4, name="woth_t")
                wotl_t = wop.tile([P, ND, NPC, 2, P], F8E5, name="wotl_t")
                nc.sync.dma_start(woth_t[:], woth_d)
                nc.sync.dma_start(wotl_t[:], wotl_d)

                # -------- Phase B: attention per head (bf16), software-
                # pipelined across heads: the next head's first two score
                # rows (and their exp) are emitted inside the current head's
                # AV tail, and the chunk-1 AV stream lags chunk-0 by one
                # k-tile so the y-normalization DVE chain never gates the
                # next head's first AV matmuls. --------
                with (
                    tc.tile_pool(name="etp", bufs=2) as etp,
                    tc.tile_pool(name="rbp", bufs=2) as rbp,
                    tc.tile_pool(name="yfp", bufs=2) as yfp,
                    tc.tile_pool(name="ps_s", bufs=2, space="PSUM") as ps_s,
                    tc.tile_pool(name="ps_ar", bufs=2, space="PSUM") as ps_ar,
                ):
                    def make_head(hl):
                        return {
                            "hl": hl,
                            "ET": etp.tile([P, NT, T], F16, tag="ET", name="ET"),
                            "rb": rbp.tile([P, T], F32, tag="rb", name="rb"),
                            "y0": ps_ar.tile([P, 512], F32, tag="y", name="y0"),
                            "r0": ps_ar.tile([P, 512], F32, tag="r", name="r0"),
                            "y1": ps_ar.tile([P, 512], F32, tag="y", name="y1"),
                            "r1": ps_ar.tile([P, 512], F32, tag="r", name="r1"),
                        }

                    def emit_ST(h, j):
                        # scores for k-tile j: per-q-chunk 1-bank psums (s0:
                        # q<512, s1: q>=512) keep the ST pipeline deep and the
                        # AV0 exp dependency short
                        hl, ET = h["hl"], h["ET"]
                        kblk = kT[:, hl, ts(j, P)]
                        if j < 4:
                            s0 = ps_s.tile([P, 512], F32, tag="s0", name="s0")
                            nc.tensor.matmul(s0[:, j * P:512], kblk,
                                             qT[:, hl, j * P:512],
                                             start=True, stop=True)
                            s1 = ps_s.tile([P, 512], F32, tag="s1", name="s1")
                            nc.tensor.matmul(s1[:], kblk, qT[:, hl, 512:T],
                                             start=True, stop=True)
                            nc.scalar.activation(ET[:, j, j * P:512],
                                                 s0[:, j * P:512], AF.Exp,
                                                 scale=float(SCALE))
                            nc.scalar.activation(ET[:, j, 512:T], s1[:],
                                                 AF.Exp, scale=float(SCALE))
                        else:
                            lo = j * P - 512
                            s1 = ps_s.tile([P, 512], F32, tag="s1", name="s1")
                            nc.tensor.matmul(s1[:, lo:512], kblk,
                                             qT[:, hl, j * P:T],
                                             start=True, stop=True)
                            nc.scalar.activation(ET[:, j, j * P:T],
                                                 s1[:, lo:512], AF.Exp,
                                                 scale=float(SCALE))
                        nc.gpsimd.affine_select(
                            out=ET[:, j, j * P:(j + 1) * P],
                            in_=ET[:, j, j * P:(j + 1) * P],
                            compare_op=OP.is_ge, fill=0.0,
                            base=0, pattern=[[1, P]], channel_multiplier=-1,
                        )

                    def emit_AV0(h, jq):  # q-chunk 0: cols jq*P..512, jq<=3
                        lo = jq * P
                        st, sp = jq == 0, jq == 3
                        ET = h["ET"]
                        nc.tensor.matmul(h["y0"][:, lo:512],
                                         vA[:, h["hl"], jq, :],
                                         ET[:, jq, lo:512], start=st, stop=sp,
                                         skip_group_check=True)
                        nc.tensor.matmul(h["r0"][:, lo:512], ones_b[:],
                                         ET[:, jq, lo:512], start=st, stop=sp,
                                         skip_group_check=True)

                    def emit_AV1(h, jq):  # q-chunk 1: cols 512..T, all jq
                        lo = max(jq * P, 512)
                        st, sp = jq == 0, jq == NT - 1
                        ET = h["ET"]
                        nc.tensor.matmul(h["y1"][:, lo - 512:512],
                                         vA[:, h["hl"], jq, :],
                                         ET[:, jq, lo:T], start=st, stop=sp,
                                         skip_group_check=True)
                        nc.tensor.matmul(h["r1"][:, lo - 512:512], ones_b[:],
                                         ET[:, jq, lo:T], start=st, stop=sp,
                                         skip_group_check=True)

                    def emit_ynorm(h, c, y, r, halves=False):
                        # yf = y * (1/r) f32; hi = e4m3(yf); lo = yf - hi
                        # (all on DVE: ACT is the busier engine in B)
                        hl, rb = h["hl"], h["rb"]
                        yf = yfp.tile([P, 512], F32, tag="yf", name="yf")
                        nsp = 1
                        w = 512 // nsp
                        for u in range(nsp):
                            cl = slice(c * 512 + u * w, c * 512 + (u + 1) * w)
                            ul = slice(u * w, (u + 1) * w)
                            nc.vector.reciprocal(rb[:, cl], r[:, ul])
                            nc.vector.tensor_mul(yf[:, ul], y[:, ul], rb[:, cl])
                            nc.vector.tensor_copy(yh[:, hl, cl], yf[:, ul])
                            nc.vector.tensor_tensor(
                                yl[:, hl, cl], yf[:, ul],
                                yh[:, hl, cl], op=OP.subtract)

                    cur = make_head(0)
                    emit_ST(cur, 0)
                    emit_ST(cur, 1)
                    emit_ST(cur, 2)
                    for hl in range(HL):
                        for j in range(2, NT):
                            if not (hl == 0 and j == 2):
                                emit_ST(cur, j)
                            if j - 2 <= 3:
                                emit_AV0(cur, j - 2)
                            if j >= 5:
                                emit_AV1(cur, j - 5)
                            if j == 5:
                                emit_ynorm(cur, 0, cur["y0"], cur["r0"])
                        emit_AV1(cur, 3)
                        emit_AV1(cur, 4)
                        if hl + 1 < HL:
                            nxt = make_head(hl + 1)
                            emit_ST(nxt, 0)
                            emit_ST(nxt, 1)
                        emit_AV1(cur, 5)
                        emit_AV1(cur, 6)
                        emit_AV1(cur, 7)
                        if hl + 1 < HL:
                            emit_ynorm(cur, 1, cur["y1"], cur["r1"])
                            cur = nxt
                        else:
                            emit_ynorm(cur, 1, cur["y1"], cur["r1"],
                                       halves=True)

                # -------- Phase C: partial out-projection (fp8 DR) --------
                with (
                    tc.tile_pool(name="osb", bufs=3) as osb,
                    tc.tile_pool(name="ps_o", bufs=3, space="PSUM") as ps_o,
                ):
                    for ob in range(ND):
                        o_ps = ps_o.tile([P, T], F32, tag="o")
                        out_sb = osb.tile([P, T], F16, tag="osb")
                        # last block's second chunk runs as two half-width
                        # psum groups so its first eviction+DMA overlaps the
                        # final matmuls (shortens the end drain)
                        subs = ([(0, 512), (512, 768), (768, 1024)]
                                if ob == ND - 1 else [(0, 512), (512, 1024)])
                        for si, (lo, hi) in enumerate(subs):
                            n = 0
                            nsub = 3 * NPC
                            for wt, yt in ((woth_t, yh), (wotl_t, yh),
                                           (woth_t, yl)):
                                for pr in range(NPC):
                                    nc.tensor.matmul(
                                        o_ps[:, lo:hi], wt[:, ob, pr],
                                        yt[:, 2 * pr:2 * pr + 2, lo:hi],
                                        start=n == 0, stop=n == nsub - 1,
                                        perf_mode=DR, skip_group_check=True)
                                    n += 1
                            # evict + DMA per sub-chunk; alternate engines
                            if si == 0:
                                nc.scalar.activation(out_sb[:, lo:hi],
                                                     o_ps[:, lo:hi], AF.Copy)
                            else:
                                nc.vector.tensor_copy(out_sb[:, lo:hi],
                                                      o_ps[:, lo:hi])
                            nc.sync.dma_start(outT_d[ob][:, lo:hi],
                                              out_sb[:, lo:hi])

    nc.compile()
    return nc


def _get_compiled():
    if "nc" not in _cache:
        _cache["nc"] = _build()
    return _cache["nc"]


def _hilo(a, lo_dt):
    import ml_dtypes
    hi = np.ascontiguousarray(a).astype(ml_dtypes.float8_e4m3)
    lo = (a - hi.astype(np.float32)).astype(lo_dt)
    return hi, lo


def _host_prep(x, Wq, bq, Wk, Wv, Wo):
    """Build per-core input maps (hi/lo fp8 splits + DR pair packing)."""
    import ml_dtypes
    E4, E5 = ml_dtypes.float8_e4m3, ml_dtypes.float8_e5m2
    in_maps = []
    xs = []
    for b in range(B):
        # [P, ND, T]: row p holds d-tile-major slices, matching the SBUF tile
        xT = np.ascontiguousarray(
            x[b].T.reshape(ND, P, T).transpose(1, 0, 2))
        xs.append(_hilo(xT, E4))
    halves = []
    for half in range(2):
        hs = slice(half * HL, (half + 1) * HL)
        # w3 packed [HL, P, NP, 2, 3E]: slot s of pair pr = d-tile 2pr+s
        w3 = np.empty((HL, P, NP, 2, 3 * E), dtype=np.float32)
        for hl, h in enumerate(range(half * HL, (half + 1) * HL)):
            for j, W in enumerate((Wq[h], Wk[h], Wv[h])):
                wt = W.T.reshape(NP, 2, P, E)          # [pr, s, p(d), e]
                w3[hl, :, :, :, j * E:(j + 1) * E] = wt.transpose(2, 0, 1, 3)
        w3h, w3l = _hilo(w3, E5)
        bqT = np.ascontiguousarray(bq[hs].T)           # [E, HL]
        # wot [ND, P, NPC, 2, P]: [ob, i-in-tile, pr, s, o], i-tile = 2pr+s
        WoT_span = Wo.T[half * 1024:(half + 1) * 1024]  # [1024, D]
        wot = np.ascontiguousarray(
            WoT_span.reshape(NPC, 2, P, ND, P).transpose(2, 3, 0, 1, 4))
        woth, wotl = _hilo(wot, E5)
        halves.append({"w3h": w3h, "w3l": w3l, "bqT": bqT,
                       "woth": woth, "wotl": wotl})
    for c in range(8):
        b, half = c // 2, c % 2
        hv = halves[half]
        in_maps.append({"xh": xs[b][0], "xl": xs[b][1], **hv})
    return in_maps


def _numpy_fallback(x, attention_mask, Wq, bq, Wk, bk, Wv, bv, Wo, bo):
    out = np.empty((B, T, D), dtype=np.float32)
    neg = np.float32(np.finfo(np.float32).min)
    for b in range(B):
        xb = x[b]
        q = np.einsum("td,hed->hte", xb, Wq) + bq[:, None, :]
        k = np.einsum("td,hed->hte", xb, Wk) + bk[:, None, :]
        v = np.einsum("td,hed->hte", xb, Wv) + bv[:, None, :]
        s = np.einsum("hqe,hke->hqk", q, k).astype(np.float32) * np.float32(SCALE)
        causal = np.arange(T)[None, :] > np.arange(T)[:, None]
        s = np.where(causal[None], neg, s)
        keep = attention_mask[b].astype(bool)
        s = np.where(keep[None, None, :], s, neg)
        s = s - s.max(-1, keepdims=True)
        p = np.exp(s)
        p = p / p.sum(-1, keepdims=True)
        y = np.einsum("hqk,hke->hqe", p, v)
        y = np.transpose(y, (1, 0, 2)).reshape(T, D)
        out[b] = y @ Wo.T + bo
    return out


def kernel(x, attention_mask, Wq, bq, Wk, bk, Wv, bv, Wo, bo):
    x = np.asarray(x, dtype=np.float32)
    attention_mask = np.asarray(attention_mask)
    Wq, bq = np.asarray(Wq, np.float32), np.asarray(bq, np.float32)
    Wk, bk = np.asarray(Wk, np.float32), np.asarray(bk, np.float32)
    Wv, bv = np.asarray(Wv, np.float32), np.asarray(bv, np.float32)
    Wo, bo = np.asarray(Wo, np.float32), np.asarray(bo, np.float32)

    if not np.all(attention_mask == 1):
        return _numpy_fallback(x, attention_mask, Wq, bq, Wk, bk, Wv, bv, Wo, bo)

    from concourse.bass_utils import run_bass_kernel_spmd

    nc = _get_compiled()
    in_maps = _host_prep(x, Wq, bq, Wk, Wv, Wo)
    res = run_bass_kernel_spmd(nc, in_maps, core_ids=list(range(8)))

    # bv folds through softmax (rows sum to 1); bk is softmax-invariant
    bo_total = (bo + Wo @ bv.reshape(D)).astype(np.float32)

    out = np.zeros((B, T, D), dtype=np.float32)
    for c in range(8):
        # fp16 partials off-device; summed here in fp32
        partial = res.results[c]["outT"].astype(np.float32).reshape(D, T)
        out[c // 2] += partial.T
    out += bo_total
    return out
